# revision 17
# baseline (speedup 1.0000x reference)
"""Distributed Trainium2 kernel for nn_AttentionHead (B=8, N=2048, H=E=1024).

Single attention head with an UPPER-triangular mask (reference masks i > j,
i.e. position i attends to j >= i), softmax over j, applied per batch:

    K = X Wk; Q = X Wq; V = X Wv
    S = Q K^T / sqrt(E);  S[i, j] = -inf for i > j
    O = softmax_j(S) V

Sharding: pure data parallel -- batch b (8) maps 1:1 onto the 8 NeuronCores.
Weights replicated; no collectives.

Per-core algorithm (v8; all matmuls bf16 with fp32 PSUM accumulation):
  - Score side folds both projections into one: S = X A X^T with
    A = Wq Wk^T, G^T = A^T X^T, so S^T tiles come from
    matmul(lhsT=X^T, rhs=G^T) -- X^T itself is the K-side operand.
  - All layout transposes (X chunks, Wq, Wk) run on the PE against a
    bf16 identity; pipelined they cost ~80ns apiece, interleaved into
    the matmul stream so HAM stays warm.
  - The attention phase is computed TRANSPOSED: exp writes P^T which is
    exactly the lhsT the PV matmul needs (no per-tile transposes of P).
    Row sums fall out of a [128,1] ones-matmul sharing the PV stationary
    weights, ordered (o0, rs, o1) so the next LDWEIGHTS hides under a
    512-wide stream.
  - Triangular structure is skipped at 128-col granularity on both the
    S^T and PV sides; the diagonal 128x128 gets an additive -1e30 mask.
  - Engine/queue split: X loads on SWDGE with in-DMA f32->bf16 cast;
    W loads f32 on the Scalar HWDGE queue; wv casts on DVE (early
    deadline), wq/wk casts on GpSimd; DVE otherwise only evacuates PSUM.
"""

import numpy as np

try:
    import concourse.bass as bass
except ImportError:  # fresh grading dir: concourse comes from the site repo
    import sys

    for p in ("/opt/trn_rl_repo", "/root/.axon_site/_ro/trn_rl_repo"):
        if p not in sys.path:
            sys.path.append(p)
    import concourse.bass as bass

import concourse.mybir as mybir
import concourse.tile as tile
from concourse import bacc, bass_utils
from concourse.masks import make_identity

B, N, H, E = 8, 2048, 1024, 1024
P = 128
HT = H // P  # 8 h-tiles
ET = E // P  # 8 e-tiles
NT = N // P  # 16 row tiles
IB = 512  # i-block width in the attention phase
NIB = N // IB  # 4
F32 = mybir.dt.float32
BF16 = mybir.dt.bfloat16
SCALE = 1.0 / float(np.sqrt(E))
NEG = -1.0e30


def build_graph():
    nc = bacc.Bacc("TRN2", target_bir_lowering=False, debug=False,
                   enable_asserts=False)
    x = nc.dram_tensor("input", [N, H], F32, kind="ExternalInput").ap()
    wk = nc.dram_tensor("k", [H, E], F32, kind="ExternalInput").ap()
    wq = nc.dram_tensor("q", [H, E], F32, kind="ExternalInput").ap()
    wv = nc.dram_tensor("v", [H, E], F32, kind="ExternalInput").ap()
    out = nc.dram_tensor("out", [N, E], F32, kind="ExternalOutput").ap()

    with tile.TileContext(nc) as tc:
        with (
            tc.tile_pool(name="const", bufs=1) as constp,
            tc.tile_pool(name="persist", bufs=1) as persist,
            tc.tile_pool(name="psMM", bufs=3, space="PSUM") as psMM,
        ):
            maskt = constp.tile([P, P], F32)
            ones = constp.tile([P, 1], BF16)
            ident16 = constp.tile([P, P], BF16)

            xt = persist.tile([P, HT, N], BF16)  # X^T [h, i]
            gt = persist.tile([P, HT, N], BF16)  # G^T [h2, i], G = X A
            vt = persist.tile([P, NT, E], BF16)  # V   [j, e]

            # ---------------- phase 1: load + project ----------------
            with (
                tc.tile_pool(name="wts", bufs=1) as wts,
                tc.tile_pool(name="stage", bufs=6) as stage,
                tc.tile_pool(name="wstage", bufs=4) as wstage,
                tc.tile_pool(name="wbst", bufs=6) as wbst,
                tc.tile_pool(name="psT", bufs=3, space="PSUM") as psT,
            ):
                wvb = wts.tile([P, HT, E], BF16, tag="wv")   # Wv natural
                wqT = wts.tile([P, ET, H], BF16, tag="wqT")  # Wq^T [e, h]
                wkT = wts.tile([P, ET, H], BF16, tag="wkT")  # Wk^T [e, h]
                ab = wts.tile([P, HT, H], BF16, tag="A")     # A [h1, h2]

                xbs = {}
                wbs = {}

                def load_x(it):
                    xb = stage.tile([P, H], BF16, tag="ld")
                    nc.gpsimd.dma_start(xb, x[it * P:(it + 1) * P, :])
                    xbs[it] = xb

                def load_wv(ho):
                    ws = wstage.tile([P, E], F32, tag="ws")
                    nc.scalar.dma_start(ws, wv[ho * P:(ho + 1) * P, :])
                    nc.vector.tensor_copy(wvb[:, ho, :], ws)

                def load_wqk(src, key, ho):
                    ws = wstage.tile([P, E], F32, tag="ws")
                    nc.scalar.dma_start(ws, src[ho * P:(ho + 1) * P, :])
                    wb = wbst.tile([P, E], BF16, tag="wb")
                    nc.gpsimd.tensor_copy(wb, ws)
                    wbs[(key, ho)] = wb

                make_identity(nc, ident16)
                for it in range(NT):
                    load_x(it)
                for ho in range(HT):
                    load_wv(ho)
                for ho in range(HT):
                    load_wqk(wq, "q", ho)
                for ho in range(HT):
                    load_wqk(wk, "k", ho)

                # mask consts (gpsimd engine; only needed by phase 2)
                nc.gpsimd.memset(maskt, 0.0)
                nc.gpsimd.affine_select(
                    out=maskt, in_=maskt, compare_op=mybir.AluOpType.is_ge,
                    fill=NEG, base=0, pattern=[[-1, P]], channel_multiplier=1,
                )
                nc.gpsimd.memset(ones, 1.0)

                def transpose_chunk(src_tile, dst, col, copy_eng=None):
                    # 8 PE transposes: src [128, 1024] -> dst[:, :, col128]
                    for ho in range(HT):
                        tp = psT.tile([P, P], BF16, tag="tp")
                        nc.tensor.transpose(
                            tp, src_tile[:, ho * P:(ho + 1) * P], ident16)
                        if copy_eng is None:
                            nc.vector.tensor_copy(
                                dst[:, ho, col * P:(col + 1) * P], tp)
                        else:
                            copy_eng(dst[:, ho, col * P:(col + 1) * P], tp)

                def v_chains(jts):
                    # V[j, e] = X Wv   (lhsT = X^T chunk)
                    for jt in jts:
                        for es in range(E // 512):
                            mm = psMM.tile([P, 512], F32, tag="mm")
                            for ho in range(HT):
                                nc.tensor.matmul(
                                    mm,
                                    lhsT=xt[:, ho, jt * P:(jt + 1) * P],
                                    rhs=wvb[:, ho, es * 512:(es + 1) * 512],
                                    start=(ho == 0), stop=(ho == HT - 1),
                                )
                            nc.vector.tensor_copy(
                                vt[:, jt, es * 512:(es + 1) * 512], mm)

                # PE order: X transposes first (they only need X chunks, so
                # the PE has work ~5us in), V chains as wv lands with the
                # remaining X and W transposes interleaved between them.
                for it in range(6):
                    transpose_chunk(xbs.pop(it), xt, it)
                for it in range(6, NT):
                    v_chains([it - 6])
                    transpose_chunk(xbs.pop(it), xt, it)
                for ho in range(4):
                    v_chains([10 + ho])
                    transpose_chunk(wbs.pop(("q", 2 * ho)), wqT, 2 * ho,
                                    nc.scalar.copy)
                    transpose_chunk(wbs.pop(("q", 2 * ho + 1)), wqT,
                                    2 * ho + 1, nc.scalar.copy)
                for ho in range(2):
                    v_chains([14 + ho])
                    for k in range(4):
                        transpose_chunk(wbs.pop(("k", 4 * ho + k)), wkT,
                                        4 * ho + k, nc.scalar.copy)

                # ---- A[h1, h2] = Wq Wk^T ----
                for h1t in range(HT):
                    for h2s in range(H // 512):
                        mm = psMM.tile([P, 512], F32, tag="mm")
                        for et in range(ET):
                            nc.tensor.matmul(
                                mm,
                                lhsT=wqT[:, et, h1t * P:(h1t + 1) * P],
                                rhs=wkT[:, et, h2s * 512:(h2s + 1) * 512],
                                start=(et == 0), stop=(et == ET - 1),
                            )
                        nc.vector.tensor_copy(
                            ab[:, h1t, h2s * 512:(h2s + 1) * 512], mm)

                # ---- GT[h2, i] = (X A)^T = A^T X^T ----
                for ns in range(N // 512):
                    for h2t in range(HT):
                        mm = psMM.tile([P, 512], F32, tag="mm")
                        for h1t in range(HT):
                            nc.tensor.matmul(
                                mm,
                                lhsT=ab[:, h1t, h2t * P:(h2t + 1) * P],
                                rhs=xt[:, h1t, ns * 512:(ns + 1) * 512],
                                start=(h1t == 0), stop=(h1t == HT - 1),
                            )
                        nc.vector.tensor_copy(
                            gt[:, h2t, ns * 512:(ns + 1) * 512], mm)

            # ---------------- phase 2: attention ----------------
            with (
                tc.tile_pool(name="ptp", bufs=18) as ptp,
                tc.tile_pool(name="obp", bufs=4) as obp,
                tc.tile_pool(name="rip", bufs=2) as rip,
                tc.tile_pool(name="psO", bufs=4, space="PSUM") as psO,
                tc.tile_pool(name="psR", bufs=1, space="PSUM") as psR,
            ):
                for ib in range(NIB):
                    i0 = ib * IB
                    ptlist = {}
                    # S^T[j, i-block] tiles, exp -> P^T
                    for j128 in range(4 * ib, NT):
                        t = j128 - 4 * ib
                        # width of the kept+diagonal region of this tile
                        w = min((t + 1) * P, IB)
                        sp = psMM.tile([P, 512], F32, tag="mm")
                        for h2t in range(HT):
                            nc.tensor.matmul(
                                sp[:, :w],
                                lhsT=xt[:, h2t, j128 * P:(j128 + 1) * P],
                                rhs=gt[:, h2t, i0:i0 + w],
                                start=(h2t == 0), stop=(h2t == HT - 1),
                            )
                        if t < 4:
                            # diagonal 128x128 sub-block gets the tri mask
                            nc.vector.tensor_add(
                                sp[:, t * P:(t + 1) * P],
                                sp[:, t * P:(t + 1) * P], maskt)
                        pt = ptp.tile([P, IB], BF16, tag="pt")
                        nc.scalar.activation(
                            pt[:, :w], sp[:, :w],
                            mybir.ActivationFunctionType.Exp,
                            bias=0.0, scale=SCALE,
                        )
                        ptlist[j128] = pt

                    # O[i, :] = P V per 128-row block; row sums via ones-MM
                    for itl in range(4):
                        it = 4 * ib + itl
                        off = itl * P
                        o0 = psO.tile([P, 512], F32, tag="o")
                        o1 = psO.tile([P, 512], F32, tag="o")
                        rs = psR.tile([P, 1], F32, tag="rs")
                        js = list(range(it, NT))
                        for m, j in enumerate(js):
                            pt = ptlist[j]
                            st = (m == 0)
                            en = (m == len(js) - 1)
                            # order (o0, rs, o1): the tiny rs matmul sits
                            # between two 512-wide streams so the next
                            # iteration's LDWEIGHTS hides under o1
                            nc.tensor.matmul(
                                o0, lhsT=pt[:, off:off + P],
                                rhs=vt[:, j, 0:512], start=st, stop=en)
                            nc.tensor.matmul(
                                rs, lhsT=pt[:, off:off + P],
                                rhs=ones, start=st, stop=en)
                            nc.tensor.matmul(
                                o1, lhsT=pt[:, off:off + P],
                                rhs=vt[:, j, 512:1024], start=st, stop=en)
                        ri = rip.tile([P, 1], F32, tag="ri")
                        nc.vector.reciprocal(ri, rs)
                        for half, op in ((0, o0), (1, o1)):
                            ob = obp.tile([P, 512], F32, tag="ob")
                            nc.scalar.mul(ob, op, ri)
                            nc.sync.dma_start(
                                out[it * P:(it + 1) * P,
                                    half * 512:(half + 1) * 512], ob)

    nc.finalize()
    return nc


_NC = None


def _get_nc():
    global _NC
    if _NC is None:
        _NC = build_graph()
    return _NC


def _run(inputs, trace=False, **kwargs):
    x = np.ascontiguousarray(np.asarray(inputs["input"], dtype=np.float32))
    k = np.ascontiguousarray(np.asarray(inputs["k"], dtype=np.float32))
    q = np.ascontiguousarray(np.asarray(inputs["q"], dtype=np.float32))
    v = np.ascontiguousarray(np.asarray(inputs["v"], dtype=np.float32))
    assert x.shape == (B, N, H)
    nc = _get_nc()
    in_maps = [
        {"input": x[b], "k": k, "q": q, "v": v} for b in range(B)
    ]
    res = bass_utils.run_bass_kernel_spmd(
        nc, in_maps, core_ids=list(range(B)), trace=trace, **kwargs)
    outs = np.stack([np.asarray(r["out"]) for r in res.results], axis=0)
    return outs.astype(np.float32), res


def kernel(**inputs):
    outs, _ = _run(inputs, trace=False)
    return outs


# revision 20
# speedup vs baseline: 1.0827x; 1.0827x over previous
"""Distributed Trainium2 kernel for nn_AttentionHead (B=8, N=2048, H=E=1024).

Single attention head with an UPPER-triangular mask (reference masks i > j,
i.e. position i attends to j >= i), softmax over j, applied per batch:

    K = X Wk; Q = X Wq; V = X Wv
    S = Q K^T / sqrt(E);  S[i, j] = -inf for i > j
    O = softmax_j(S) V

Sharding: pure data parallel -- batch b (8) maps 1:1 onto the 8 NeuronCores.
Weights replicated; no collectives.

Per-core algorithm (v8; all matmuls bf16 with fp32 PSUM accumulation):
  - Score side folds both projections into one: S = X A X^T with
    A = Wq Wk^T, G^T = A^T X^T, so S^T tiles come from
    matmul(lhsT=X^T, rhs=G^T) -- X^T itself is the K-side operand.
  - All layout transposes (X chunks, Wq, Wk) run on the PE against a
    bf16 identity; pipelined they cost ~80ns apiece, interleaved into
    the matmul stream so HAM stays warm.
  - The attention phase is computed TRANSPOSED: exp writes P^T which is
    exactly the lhsT the PV matmul needs (no per-tile transposes of P).
    Row sums fall out of a [128,1] ones-matmul sharing the PV stationary
    weights, ordered (o0, rs, o1) so the next LDWEIGHTS hides under a
    512-wide stream.
  - Triangular structure is skipped at 128-col granularity on both the
    S^T and PV sides; the diagonal 128x128 gets an additive -1e30 mask.
  - Engine/queue split: X loads on SWDGE with in-DMA f32->bf16 cast;
    W loads f32 on the Scalar HWDGE queue; wv casts on DVE (early
    deadline), wq/wk casts on GpSimd; DVE otherwise only evacuates PSUM.
"""

import numpy as np

try:
    import concourse.bass as bass
except ImportError:  # fresh grading dir: concourse comes from the site repo
    import sys

    for p in ("/opt/trn_rl_repo", "/root/.axon_site/_ro/trn_rl_repo"):
        if p not in sys.path:
            sys.path.append(p)
    import concourse.bass as bass

import concourse.mybir as mybir
import concourse.tile as tile
from concourse import bacc, bass_utils
from concourse.masks import make_identity

B, N, H, E = 8, 2048, 1024, 1024
P = 128
HT = H // P  # 8 h-tiles
ET = E // P  # 8 e-tiles
NT = N // P  # 16 row tiles
IB = 512  # i-block width in the attention phase
NIB = N // IB  # 4
F32 = mybir.dt.float32
BF16 = mybir.dt.bfloat16
SCALE = 1.0 / float(np.sqrt(E))
NEG = -1.0e30


def build_graph():
    nc = bacc.Bacc("TRN2", target_bir_lowering=False, debug=False,
                   enable_asserts=False)
    x = nc.dram_tensor("input", [N, H], F32, kind="ExternalInput").ap()
    wk = nc.dram_tensor("k", [H, E], F32, kind="ExternalInput").ap()
    wq = nc.dram_tensor("q", [H, E], F32, kind="ExternalInput").ap()
    wv = nc.dram_tensor("v", [H, E], F32, kind="ExternalInput").ap()
    out = nc.dram_tensor("out", [N, E], F32, kind="ExternalOutput").ap()

    with tile.TileContext(nc) as tc:
        with (
            tc.tile_pool(name="const", bufs=1) as constp,
            tc.tile_pool(name="persist", bufs=1) as persist,
            tc.tile_pool(name="psMM", bufs=3, space="PSUM") as psMM,
        ):
            maskt = constp.tile([P, P], F32)
            ones = constp.tile([P, 1], BF16)
            ident16 = constp.tile([P, P], BF16)

            xt = persist.tile([P, HT, N], BF16)  # X^T [h, i]
            gt = persist.tile([P, HT, N], BF16)  # G^T [h2, i], G = X A
            vt = persist.tile([P, NT, E], BF16)  # V   [j, e]

            # ---------------- phase 1: load + project ----------------
            with (
                tc.tile_pool(name="wts", bufs=1) as wts,
                tc.tile_pool(name="stage", bufs=6) as stage,
                tc.tile_pool(name="wstage", bufs=4) as wstage,
                tc.tile_pool(name="wbst", bufs=6) as wbst,
                tc.tile_pool(name="psT", bufs=3, space="PSUM") as psT,
            ):
                wvb = wts.tile([P, HT, E], BF16, tag="wv")   # Wv natural
                wqT = wts.tile([P, ET, H], BF16, tag="wqT")  # Wq^T [e, h]
                wkT = wts.tile([P, ET, H], BF16, tag="wkT")  # Wk^T [e, h]
                ab = wts.tile([P, HT, H], BF16, tag="A")     # A [h1, h2]

                xbs = {}
                wbs = {}

                def load_x(it):
                    xb = stage.tile([P, H], BF16, tag="ld")
                    nc.gpsimd.dma_start(xb, x[it * P:(it + 1) * P, :])
                    xbs[it] = xb

                def load_x_fast(it):
                    # HWDGE f32 + DVE cast: lands ~6us earlier than the
                    # SWDGE path; used for the first chunks so the PE's
                    # transpose work starts as soon as possible
                    xf = wstage.tile([P, E], F32, tag="ws")
                    nc.sync.dma_start(xf, x[it * P:(it + 1) * P, :])
                    xb = stage.tile([P, H], BF16, tag="ld")
                    nc.vector.tensor_copy(xb, xf)
                    xbs[it] = xb

                def load_wv(ho):
                    ws = wstage.tile([P, E], F32, tag="ws")
                    nc.scalar.dma_start(ws, wv[ho * P:(ho + 1) * P, :])
                    nc.vector.tensor_copy(wvb[:, ho, :], ws)

                def load_wqk(src, key, ho):
                    ws = wstage.tile([P, E], F32, tag="ws")
                    nc.scalar.dma_start(ws, src[ho * P:(ho + 1) * P, :])
                    wb = wbst.tile([P, E], BF16, tag="wb")
                    nc.gpsimd.tensor_copy(wb, ws)
                    wbs[(key, ho)] = wb

                make_identity(nc, ident16)
                for it in range(2):
                    load_x_fast(it)
                for it in range(2, NT):
                    load_x(it)
                for ho in range(HT):
                    load_wv(ho)
                for ho in range(HT):
                    load_wqk(wq, "q", ho)
                for ho in range(HT):
                    load_wqk(wk, "k", ho)

                # mask consts (gpsimd engine; only needed by phase 2)
                nc.gpsimd.memset(maskt, 0.0)
                nc.gpsimd.affine_select(
                    out=maskt, in_=maskt, compare_op=mybir.AluOpType.is_ge,
                    fill=NEG, base=0, pattern=[[-1, P]], channel_multiplier=1,
                )
                nc.gpsimd.memset(ones, 1.0)

                def transpose_chunk(src_tile, dst, col, copy_eng=None):
                    # 8 PE transposes: src [128, 1024] -> dst[:, :, col128]
                    for ho in range(HT):
                        tp = psT.tile([P, P], BF16, tag="tp")
                        nc.tensor.transpose(
                            tp, src_tile[:, ho * P:(ho + 1) * P], ident16)
                        if copy_eng is None:
                            nc.vector.tensor_copy(
                                dst[:, ho, col * P:(col + 1) * P], tp)
                        else:
                            copy_eng(dst[:, ho, col * P:(col + 1) * P], tp)

                def v_chains(jts):
                    # V[j, e] = X Wv   (lhsT = X^T chunk)
                    for jt in jts:
                        for es in range(E // 512):
                            mm = psMM.tile([P, 512], F32, tag="mm")
                            for ho in range(HT):
                                nc.tensor.matmul(
                                    mm,
                                    lhsT=xt[:, ho, jt * P:(jt + 1) * P],
                                    rhs=wvb[:, ho, es * 512:(es + 1) * 512],
                                    start=(ho == 0), stop=(ho == HT - 1),
                                )
                            nc.vector.tensor_copy(
                                vt[:, jt, es * 512:(es + 1) * 512], mm)

                # PE order: X transposes first (they only need X chunks, so
                # the PE has work ~5us in), V chains as wv lands with the
                # remaining X and W transposes interleaved between them.
                for it in range(6):
                    transpose_chunk(xbs.pop(it), xt, it)
                for it in range(6, NT):
                    v_chains([it - 6])
                    transpose_chunk(xbs.pop(it), xt, it)
                for ho in range(4):
                    v_chains([10 + ho])
                    transpose_chunk(wbs.pop(("q", 2 * ho)), wqT, 2 * ho)
                    transpose_chunk(wbs.pop(("q", 2 * ho + 1)), wqT,
                                    2 * ho + 1)
                for ho in range(2):
                    v_chains([14 + ho])
                    for k in range(4):
                        transpose_chunk(wbs.pop(("k", 4 * ho + k)), wkT,
                                        4 * ho + k)

                # ---- A[h1, h2] = Wq Wk^T ----
                for h1t in range(HT):
                    for h2s in range(H // 512):
                        mm = psMM.tile([P, 512], F32, tag="mm")
                        for et in range(ET):
                            nc.tensor.matmul(
                                mm,
                                lhsT=wqT[:, et, h1t * P:(h1t + 1) * P],
                                rhs=wkT[:, et, h2s * 512:(h2s + 1) * 512],
                                start=(et == 0), stop=(et == ET - 1),
                            )
                        nc.vector.tensor_copy(
                            ab[:, h1t, h2s * 512:(h2s + 1) * 512], mm)

                # ---- GT[h2, i] = (X A)^T = A^T X^T ----
                for ns in range(N // 512):
                    for h2t in range(HT):
                        mm = psMM.tile([P, 512], F32, tag="mm")
                        for h1t in range(HT):
                            nc.tensor.matmul(
                                mm,
                                lhsT=ab[:, h1t, h2t * P:(h2t + 1) * P],
                                rhs=xt[:, h1t, ns * 512:(ns + 1) * 512],
                                start=(h1t == 0), stop=(h1t == HT - 1),
                            )
                        nc.vector.tensor_copy(
                            gt[:, h2t, ns * 512:(ns + 1) * 512], mm)

            # ---------------- phase 2: attention ----------------
            with (
                tc.tile_pool(name="ptp", bufs=18) as ptp,
                tc.tile_pool(name="obp", bufs=4) as obp,
                tc.tile_pool(name="rip", bufs=2) as rip,
                tc.tile_pool(name="psO", bufs=4, space="PSUM") as psO,
                tc.tile_pool(name="psR", bufs=1, space="PSUM") as psR,
            ):
                for ib in range(NIB):
                    i0 = ib * IB
                    ptlist = {}
                    # S^T[j, i-block] tiles, exp -> P^T
                    for j128 in range(4 * ib, NT):
                        t = j128 - 4 * ib
                        # width of the kept+diagonal region of this tile
                        w = min((t + 1) * P, IB)
                        sp = psMM.tile([P, 512], F32, tag="mm")
                        for h2t in range(HT):
                            nc.tensor.matmul(
                                sp[:, :w],
                                lhsT=xt[:, h2t, j128 * P:(j128 + 1) * P],
                                rhs=gt[:, h2t, i0:i0 + w],
                                start=(h2t == 0), stop=(h2t == HT - 1),
                            )
                        if t < 4:
                            # diagonal 128x128 sub-block gets the tri mask
                            nc.vector.tensor_add(
                                sp[:, t * P:(t + 1) * P],
                                sp[:, t * P:(t + 1) * P], maskt)
                        pt = ptp.tile([P, IB], BF16, tag="pt")
                        nc.scalar.activation(
                            pt[:, :w], sp[:, :w],
                            mybir.ActivationFunctionType.Exp,
                            bias=0.0, scale=SCALE,
                        )
                        ptlist[j128] = pt

                    # O[i, :] = P V per 128-row block; row sums via ones-MM
                    for itl in range(4):
                        it = 4 * ib + itl
                        off = itl * P
                        o0 = psO.tile([P, 512], F32, tag="o")
                        o1 = psO.tile([P, 512], F32, tag="o")
                        rs = psR.tile([P, 1], F32, tag="rs")
                        js = list(range(it, NT))
                        for m, j in enumerate(js):
                            pt = ptlist[j]
                            st = (m == 0)
                            en = (m == len(js) - 1)
                            # order (o0, rs, o1): the tiny rs matmul sits
                            # between two 512-wide streams so the next
                            # iteration's LDWEIGHTS hides under o1
                            nc.tensor.matmul(
                                o0, lhsT=pt[:, off:off + P],
                                rhs=vt[:, j, 0:512], start=st, stop=en)
                            nc.tensor.matmul(
                                rs, lhsT=pt[:, off:off + P],
                                rhs=ones, start=st, stop=en)
                            nc.tensor.matmul(
                                o1, lhsT=pt[:, off:off + P],
                                rhs=vt[:, j, 512:1024], start=st, stop=en)
                        ri = rip.tile([P, 1], F32, tag="ri")
                        nc.vector.reciprocal(ri, rs)
                        for half, op in ((0, o0), (1, o1)):
                            ob = obp.tile([P, 512], F32, tag="ob")
                            nc.scalar.mul(ob, op, ri)
                            nc.sync.dma_start(
                                out[it * P:(it + 1) * P,
                                    half * 512:(half + 1) * 512], ob)

    nc.finalize()
    return nc


_NC = None


def _get_nc():
    global _NC
    if _NC is None:
        _NC = build_graph()
    return _NC


def _run(inputs, trace=False, **kwargs):
    x = np.ascontiguousarray(np.asarray(inputs["input"], dtype=np.float32))
    k = np.ascontiguousarray(np.asarray(inputs["k"], dtype=np.float32))
    q = np.ascontiguousarray(np.asarray(inputs["q"], dtype=np.float32))
    v = np.ascontiguousarray(np.asarray(inputs["v"], dtype=np.float32))
    assert x.shape == (B, N, H)
    nc = _get_nc()
    in_maps = [
        {"input": x[b], "k": k, "q": q, "v": v} for b in range(B)
    ]
    res = bass_utils.run_bass_kernel_spmd(
        nc, in_maps, core_ids=list(range(B)), trace=trace, **kwargs)
    outs = np.stack([np.asarray(r["out"]) for r in res.results], axis=0)
    return outs.astype(np.float32), res


def kernel(**inputs):
    outs, _ = _run(inputs, trace=False)
    return outs


# revision 22
# speedup vs baseline: 1.1194x; 1.0340x over previous
"""Distributed Trainium2 kernel for nn_AttentionHead (B=8, N=2048, H=E=1024).

Single attention head with an UPPER-triangular mask (reference masks i > j,
i.e. position i attends to j >= i), softmax over j, applied per batch:

    K = X Wk; Q = X Wq; V = X Wv
    S = Q K^T / sqrt(E);  S[i, j] = -inf for i > j
    O = softmax_j(S) V

Sharding: pure data parallel -- batch b (8) maps 1:1 onto the 8 NeuronCores.
Weights replicated; no collectives.

Per-core algorithm (v8; all matmuls bf16 with fp32 PSUM accumulation):
  - Score side folds both projections into one: S = X A X^T with
    A = Wq Wk^T, G^T = A^T X^T, so S^T tiles come from
    matmul(lhsT=X^T, rhs=G^T) -- X^T itself is the K-side operand.
  - All layout transposes (X chunks, Wq, Wk) run on the PE against a
    bf16 identity; pipelined they cost ~80ns apiece, interleaved into
    the matmul stream so HAM stays warm.
  - The attention phase is computed TRANSPOSED: exp writes P^T which is
    exactly the lhsT the PV matmul needs (no per-tile transposes of P).
    Row sums fall out of a [128,1] ones-matmul sharing the PV stationary
    weights, ordered (o0, rs, o1) so the next LDWEIGHTS hides under a
    512-wide stream.
  - Triangular structure is skipped at 128-col granularity on both the
    S^T and PV sides; the diagonal 128x128 gets an additive -1e30 mask.
  - Engine/queue split: X loads on SWDGE with in-DMA f32->bf16 cast;
    W loads f32 on the Scalar HWDGE queue; wv casts on DVE (early
    deadline), wq/wk casts on GpSimd; DVE otherwise only evacuates PSUM.
"""

import numpy as np

try:
    import concourse.bass as bass
except ImportError:  # fresh grading dir: concourse comes from the site repo
    import sys

    for p in ("/opt/trn_rl_repo", "/root/.axon_site/_ro/trn_rl_repo"):
        if p not in sys.path:
            sys.path.append(p)
    import concourse.bass as bass

import concourse.mybir as mybir
import concourse.tile as tile
from concourse import bacc, bass_utils
from concourse.masks import make_identity

B, N, H, E = 8, 2048, 1024, 1024
P = 128
HT = H // P  # 8 h-tiles
ET = E // P  # 8 e-tiles
NT = N // P  # 16 row tiles
IB = 512  # i-block width in the attention phase
NIB = N // IB  # 4
F32 = mybir.dt.float32
BF16 = mybir.dt.bfloat16
SCALE = 1.0 / float(np.sqrt(E))
NEG = -1.0e30


def build_graph():
    nc = bacc.Bacc("TRN2", target_bir_lowering=False, debug=False,
                   enable_asserts=False)
    x = nc.dram_tensor("input", [N, H], F32, kind="ExternalInput").ap()
    wk = nc.dram_tensor("k", [H, E], F32, kind="ExternalInput").ap()
    wq = nc.dram_tensor("q", [H, E], F32, kind="ExternalInput").ap()
    wv = nc.dram_tensor("v", [H, E], F32, kind="ExternalInput").ap()
    out = nc.dram_tensor("out", [N, E], F32, kind="ExternalOutput").ap()

    with tile.TileContext(nc) as tc:
        with (
            tc.tile_pool(name="const", bufs=1) as constp,
            tc.tile_pool(name="persist", bufs=1) as persist,
            tc.tile_pool(name="psMM", bufs=3, space="PSUM") as psMM,
        ):
            maskt = constp.tile([P, P], F32)
            ones = constp.tile([P, 1], BF16)
            ident16 = constp.tile([P, P], BF16)

            xt = persist.tile([P, HT, N], BF16)  # X^T [h, i]
            gt = persist.tile([P, HT, N], BF16)  # G^T [h2, i], G = X A
            vt = persist.tile([P, NT, E], BF16)  # V   [j, e]

            # ---------------- phase 1: load + project ----------------
            with (
                tc.tile_pool(name="wts", bufs=1) as wts,
                tc.tile_pool(name="stage", bufs=6) as stage,
                tc.tile_pool(name="wstage", bufs=4) as wstage,
                tc.tile_pool(name="wbst", bufs=6) as wbst,
                tc.tile_pool(name="psT", bufs=3, space="PSUM") as psT,
            ):
                wvb = wts.tile([P, HT, E], BF16, tag="wv")   # Wv natural
                wqT = wts.tile([P, ET, H], BF16, tag="wqT")  # Wq^T [e, h]
                wkT = wts.tile([P, ET, H], BF16, tag="wkT")  # Wk^T [e, h]
                ab = wts.tile([P, HT, H], BF16, tag="A")     # A [h1, h2]

                xbs = {}
                wbs = {}

                def load_x(it):
                    xb = stage.tile([P, H], BF16, tag="ld")
                    nc.gpsimd.dma_start(xb, x[it * P:(it + 1) * P, :])
                    xbs[it] = xb

                def load_x_fast(it):
                    # HWDGE f32 + DVE cast: lands ~6us earlier than the
                    # SWDGE path; used for the first chunks so the PE's
                    # transpose work starts as soon as possible
                    xf = wstage.tile([P, E], F32, tag="ws")
                    nc.sync.dma_start(xf, x[it * P:(it + 1) * P, :])
                    xb = stage.tile([P, H], BF16, tag="ld")
                    nc.vector.tensor_copy(xb, xf)
                    xbs[it] = xb

                def load_wv(ho):
                    ws = wstage.tile([P, E], F32, tag="ws")
                    nc.scalar.dma_start(ws, wv[ho * P:(ho + 1) * P, :])
                    nc.vector.tensor_copy(wvb[:, ho, :], ws)

                def load_wqk(src, key, ho):
                    ws = wstage.tile([P, E], F32, tag="ws")
                    nc.scalar.dma_start(ws, src[ho * P:(ho + 1) * P, :])
                    wb = wbst.tile([P, E], BF16, tag="wb")
                    nc.gpsimd.tensor_copy(wb, ws)
                    wbs[(key, ho)] = wb

                make_identity(nc, ident16)
                for it in range(2):
                    load_x_fast(it)
                for it in range(2, NT):
                    load_x(it)
                for ho in range(HT):
                    load_wv(ho)
                for ho in range(HT):
                    load_wqk(wk, "k", ho)
                for ho in range(HT):
                    load_wqk(wq, "q", ho)

                # mask consts (gpsimd engine; only needed by phase 2)
                nc.gpsimd.memset(maskt, 0.0)
                nc.gpsimd.affine_select(
                    out=maskt, in_=maskt, compare_op=mybir.AluOpType.is_ge,
                    fill=NEG, base=0, pattern=[[-1, P]], channel_multiplier=1,
                )
                nc.gpsimd.memset(ones, 1.0)

                def transpose_chunk(src_tile, dst, col, copy_eng=None):
                    # 8 PE transposes: src [128, 1024] -> dst[:, :, col128]
                    for ho in range(HT):
                        tp = psT.tile([P, P], BF16, tag="tp")
                        nc.tensor.transpose(
                            tp, src_tile[:, ho * P:(ho + 1) * P], ident16)
                        if copy_eng is None:
                            nc.vector.tensor_copy(
                                dst[:, ho, col * P:(col + 1) * P], tp)
                        else:
                            copy_eng(dst[:, ho, col * P:(col + 1) * P], tp)

                def v_chains(jts):
                    # V[j, e] = X Wv   (lhsT = X^T chunk)
                    for jt in jts:
                        for es in range(E // 512):
                            mm = psMM.tile([P, 512], F32, tag="mm")
                            for ho in range(HT):
                                nc.tensor.matmul(
                                    mm,
                                    lhsT=xt[:, ho, jt * P:(jt + 1) * P],
                                    rhs=wvb[:, ho, es * 512:(es + 1) * 512],
                                    start=(ho == 0), stop=(ho == HT - 1),
                                )
                            nc.vector.tensor_copy(
                                vt[:, jt, es * 512:(es + 1) * 512], mm)

                # PE order: X transposes first (they only need X chunks, so
                # the PE has work ~5us in), V chains as wv lands with the
                # remaining X and W transposes interleaved between them.
                for it in range(6):
                    transpose_chunk(xbs.pop(it), xt, it)
                for it in range(6, NT):
                    v_chains([it - 6])
                    transpose_chunk(xbs.pop(it), xt, it)
                # wk transposes first: every A chain's rhs needs wkT, while
                # wqT chunk h1t is only needed once A row-block h1t starts
                for ho in range(4):
                    v_chains([10 + ho])
                    transpose_chunk(wbs.pop(("k", 2 * ho)), wkT, 2 * ho)
                    transpose_chunk(wbs.pop(("k", 2 * ho + 1)), wkT,
                                    2 * ho + 1)
                for ho in range(2):
                    v_chains([14 + ho])
                    for k in range(4):
                        transpose_chunk(wbs.pop(("q", 4 * ho + k)), wqT,
                                        4 * ho + k)

                # ---- A[h1, h2] = Wq Wk^T ----
                for h1t in range(HT):
                    for h2s in range(H // 512):
                        mm = psMM.tile([P, 512], F32, tag="mm")
                        for et in range(ET):
                            nc.tensor.matmul(
                                mm,
                                lhsT=wqT[:, et, h1t * P:(h1t + 1) * P],
                                rhs=wkT[:, et, h2s * 512:(h2s + 1) * 512],
                                start=(et == 0), stop=(et == ET - 1),
                            )
                        nc.vector.tensor_copy(
                            ab[:, h1t, h2s * 512:(h2s + 1) * 512], mm)

                # ---- GT[h2, i] = (X A)^T = A^T X^T ----
                for ns in range(N // 512):
                    for h2t in range(HT):
                        mm = psMM.tile([P, 512], F32, tag="mm")
                        for h1t in range(HT):
                            nc.tensor.matmul(
                                mm,
                                lhsT=ab[:, h1t, h2t * P:(h2t + 1) * P],
                                rhs=xt[:, h1t, ns * 512:(ns + 1) * 512],
                                start=(h1t == 0), stop=(h1t == HT - 1),
                            )
                        nc.vector.tensor_copy(
                            gt[:, h2t, ns * 512:(ns + 1) * 512], mm)

            # ---------------- phase 2: attention ----------------
            with (
                tc.tile_pool(name="ptp", bufs=18) as ptp,
                tc.tile_pool(name="obp", bufs=4) as obp,
                tc.tile_pool(name="rip", bufs=2) as rip,
                tc.tile_pool(name="psO", bufs=4, space="PSUM") as psO,
                tc.tile_pool(name="psR", bufs=1, space="PSUM") as psR,
            ):
                for ib in range(NIB):
                    i0 = ib * IB
                    ptlist = {}
                    # S^T[j, i-block] tiles, exp -> P^T
                    for j128 in range(4 * ib, NT):
                        t = j128 - 4 * ib
                        # width of the kept+diagonal region of this tile
                        w = min((t + 1) * P, IB)
                        sp = psMM.tile([P, 512], F32, tag="mm")
                        for h2t in range(HT):
                            nc.tensor.matmul(
                                sp[:, :w],
                                lhsT=xt[:, h2t, j128 * P:(j128 + 1) * P],
                                rhs=gt[:, h2t, i0:i0 + w],
                                start=(h2t == 0), stop=(h2t == HT - 1),
                            )
                        if t < 4:
                            # diagonal 128x128 sub-block gets the tri mask
                            nc.vector.tensor_add(
                                sp[:, t * P:(t + 1) * P],
                                sp[:, t * P:(t + 1) * P], maskt)
                        pt = ptp.tile([P, IB], BF16, tag="pt")
                        nc.scalar.activation(
                            pt[:, :w], sp[:, :w],
                            mybir.ActivationFunctionType.Exp,
                            bias=0.0, scale=SCALE,
                        )
                        ptlist[j128] = pt

                    # O[i, :] = P V per 128-row block; row sums via ones-MM
                    for itl in range(4):
                        it = 4 * ib + itl
                        off = itl * P
                        o0 = psO.tile([P, 512], F32, tag="o")
                        o1 = psO.tile([P, 512], F32, tag="o")
                        rs = psR.tile([P, 1], F32, tag="rs")
                        js = list(range(it, NT))
                        for m, j in enumerate(js):
                            pt = ptlist[j]
                            st = (m == 0)
                            en = (m == len(js) - 1)
                            # order (o0, rs, o1): the tiny rs matmul sits
                            # between two 512-wide streams so the next
                            # iteration's LDWEIGHTS hides under o1
                            nc.tensor.matmul(
                                o0, lhsT=pt[:, off:off + P],
                                rhs=vt[:, j, 0:512], start=st, stop=en)
                            nc.tensor.matmul(
                                rs, lhsT=pt[:, off:off + P],
                                rhs=ones, start=st, stop=en)
                            nc.tensor.matmul(
                                o1, lhsT=pt[:, off:off + P],
                                rhs=vt[:, j, 512:1024], start=st, stop=en)
                        ri = rip.tile([P, 1], F32, tag="ri")
                        nc.vector.reciprocal(ri, rs)
                        for half, op in ((0, o0), (1, o1)):
                            ob = obp.tile([P, 512], F32, tag="ob")
                            nc.scalar.mul(ob, op, ri)
                            nc.sync.dma_start(
                                out[it * P:(it + 1) * P,
                                    half * 512:(half + 1) * 512], ob)

    nc.finalize()
    return nc


_NC = None


def _get_nc():
    global _NC
    if _NC is None:
        _NC = build_graph()
    return _NC


def _run(inputs, trace=False, **kwargs):
    x = np.ascontiguousarray(np.asarray(inputs["input"], dtype=np.float32))
    k = np.ascontiguousarray(np.asarray(inputs["k"], dtype=np.float32))
    q = np.ascontiguousarray(np.asarray(inputs["q"], dtype=np.float32))
    v = np.ascontiguousarray(np.asarray(inputs["v"], dtype=np.float32))
    assert x.shape == (B, N, H)
    nc = _get_nc()
    in_maps = [
        {"input": x[b], "k": k, "q": q, "v": v} for b in range(B)
    ]
    res = bass_utils.run_bass_kernel_spmd(
        nc, in_maps, core_ids=list(range(B)), trace=trace, **kwargs)
    outs = np.stack([np.asarray(r["out"]) for r in res.results], axis=0)
    return outs.astype(np.float32), res


def kernel(**inputs):
    outs, _ = _run(inputs, trace=False)
    return outs
